# revision 1
# baseline (speedup 1.0000x reference)
"""Mixtral sparse-MoE block on 8 TRN2 NeuronCores, expert-parallel.

Strategy (per sharding hint): the E=8 experts are sharded 1:1 across the 8
cores; the router is evaluated on host (tiny: [8192,1024]x[1024,8]) and each
core runs its expert's SwiGLU MLP on the tokens routed to it (top-2 routing,
capacity C per expert), scaling by the renormalized routing weight on-device.
The host scatter-adds the per-expert partials (the "psum of the combine").

Device math is bf16 matmul with fp32 PSUM accumulation; per-token expert
outputs are bitwise-independent of the batch composition, so gathered
(sparse) execution matches dense-then-mask execution numerically.
"""

import os
import sys

import numpy as np
import ml_dtypes

for _p in ("/opt/trn_rl_repo", "/root/.axon_site/_ro/trn_rl_repo"):
    if os.path.isdir(_p) and _p not in sys.path:
        sys.path.append(_p)

import concourse.mybir as mybir
import concourse.tile as tile
from concourse import bacc
from concourse.bass import ts
from concourse.bass_utils import run_bass_kernel_spmd

BF16 = ml_dtypes.bfloat16

B, S, D, F, E = 2, 4096, 1024, 3584, 8
T = B * S
TOK = 512            # token tile (one PSUM bank of fp32)
ND = D // 128        # 8 contraction chunks for up/gate
NF = F // 128        # 28 contraction chunks for down
CAP = 2560           # per-expert token capacity (max count ~2182 at 8.3% over-mean)
DENSE = False        # True: every core runs all T tokens (no gather)

TRACE = False        # set by test.py to collect an NTFF profile
TRACE_CORES = None
LAST = {}            # test.py reads LAST["results"] (BassKernelResults)

_CACHE = {}


def _build(n_tok):
    """One-core Bass program: SwiGLU expert MLP over n_tok gathered tokens.

    Inputs (per core): ht [128,ND,n_tok] bf16  = h^T (D-major partition tiles)
                       wu/wg [128,ND,F] bf16   = w_up/w_gate (D on partitions)
                       wd [ND,128,NF,128] bf16 = w_down as [d_chunk][f_in][f_chunk][d_in]
                       cc [128,n_tok] f32      = routing weight per token, row-broadcast
    Output: out [128,ND,n_tok] f32 = (weighted expert output)^T
    """
    nc = bacc.Bacc("TRN2", target_bir_lowering=False, debug=False, num_devices=E)
    f32, bf16 = mybir.dt.float32, mybir.dt.bfloat16
    ht_d = nc.dram_tensor("ht", [128, ND, n_tok], bf16, kind="ExternalInput").ap()
    wu_d = nc.dram_tensor("wu", [128, ND, F], bf16, kind="ExternalInput").ap()
    wg_d = nc.dram_tensor("wg", [128, ND, F], bf16, kind="ExternalInput").ap()
    wd_d = nc.dram_tensor("wd", [ND, 128, NF, 128], bf16, kind="ExternalInput").ap()
    cc_d = nc.dram_tensor("cc", [128, n_tok], f32, kind="ExternalInput").ap()
    out_d = nc.dram_tensor("out", [128, ND, n_tok], f32, kind="ExternalOutput").ap()

    n_tiles = n_tok // TOK
    with tile.TileContext(nc) as tc:
        with (
            tc.tile_pool(name="wconst", bufs=1) as wpool,
            tc.tile_pool(name="htp", bufs=2) as htpool,
            tc.tile_pool(name="wdp", bufs=2) as wdpool,
            tc.tile_pool(name="actp", bufs=1) as actpool,
            tc.tile_pool(name="silp", bufs=2) as silpool,
            tc.tile_pool(name="outp", bufs=3) as outpool,
            tc.tile_pool(name="ps", bufs=2, space="PSUM") as pspool,
        ):
            wu = wpool.tile([128, ND, F], bf16, tag="wu")
            nc.sync.dma_start(wu, wu_d)
            wg = wpool.tile([128, ND, F], bf16, tag="wg")
            nc.sync.dma_start(wg, wg_d)
            cc = wpool.tile([128, n_tok], f32, tag="cc")
            nc.sync.dma_start(cc, cc_d)

            for t in range(n_tiles):
                ht = htpool.tile([128, ND, TOK], bf16, tag="ht")
                nc.sync.dma_start(ht, ht_d[:, :, ts(t, TOK)])
                act = actpool.tile([128, NF, TOK], bf16, tag="act")
                for f in range(NF):
                    up_ps = pspool.tile([128, TOK], f32, tag="up")
                    gp_ps = pspool.tile([128, TOK], f32, tag="gp")
                    for d in range(ND):
                        nc.tensor.matmul(up_ps, wu[:, d, ts(f, 128)], ht[:, d, :],
                                         start=(d == 0), stop=(d == ND - 1))
                    for d in range(ND):
                        nc.tensor.matmul(gp_ps, wg[:, d, ts(f, 128)], ht[:, d, :],
                                         start=(d == 0), stop=(d == ND - 1))
                    sil = silpool.tile([128, TOK], f32, tag="sil")
                    nc.scalar.activation(sil, up_ps,
                                         mybir.ActivationFunctionType.Silu)
                    nc.vector.tensor_mul(act[:, f, :], sil, gp_ps)
                for po in range(ND):
                    wdt = wdpool.tile([128, NF, 128], bf16, tag="wd")
                    nc.sync.dma_start(wdt, wd_d[po])
                    o_ps = pspool.tile([128, TOK], f32, tag="o")
                    for f in range(NF):
                        nc.tensor.matmul(o_ps, wdt[:, f, :], act[:, f, :],
                                         start=(f == 0), stop=(f == NF - 1))
                    ob = outpool.tile([128, TOK], f32, tag="ob")
                    nc.vector.tensor_mul(ob, o_ps, cc[:, ts(t, TOK)])
                    nc.sync.dma_start(out_d[:, po, ts(t, TOK)], ob)
    nc.compile()
    return nc


def _get_nc(n_tok):
    if n_tok not in _CACHE:
        _CACHE[n_tok] = _build(n_tok)
    return _CACHE[n_tok]


def kernel(hidden_states, w_router, w_up, w_gate, w_down):
    h = np.asarray(hidden_states, np.float32).reshape(T, D)
    wr = np.asarray(w_router, np.float32)
    w_up = np.asarray(w_up, np.float32)
    w_gate = np.asarray(w_gate, np.float32)
    w_down = np.asarray(w_down, np.float32)

    # Router on host (fp32, same math as reference).
    logits = h @ wr                                            # [T, E]
    p = logits - logits.max(1, keepdims=True)
    np.exp(p, out=p)
    p /= p.sum(1, keepdims=True)
    sel = np.argsort(-p, axis=1, kind="stable")[:, :2]         # lax.top_k ties
    tix = np.arange(T)
    w1 = p[tix, sel[:, 0]]
    w2 = p[tix, sel[:, 1]]
    denom = w1 + w2
    rw1 = (w1 / denom).astype(np.float32)
    rw2 = (w2 / denom).astype(np.float32)

    # h^T in bf16, tiled [128, ND, T] (partition-inner on D).
    hT16 = np.ascontiguousarray(h.T).astype(BF16)              # [D, T]
    hT16 = hT16.reshape(ND, 128, T).transpose(1, 0, 2)         # [128, ND, T]

    n_tok = T if DENSE else CAP
    nc = _get_nc(n_tok)

    in_maps = []
    idx_list = []
    for e in range(E):
        if DENSE:
            idx = tix
            wtok = np.where(sel[:, 0] == e, rw1,
                            np.where(sel[:, 1] == e, rw2, 0.0)).astype(np.float32)
        else:
            m = (sel[:, 0] == e) | (sel[:, 1] == e)
            idx = np.nonzero(m)[0]
            assert len(idx) <= CAP, f"expert {e} overflow: {len(idx)} > {CAP}"
            wtok = np.where(sel[:, 0] == e, rw1, rw2)[idx].astype(np.float32)
        idx_list.append(idx)

        ht_e = np.zeros((128, ND, n_tok), BF16)
        ht_e[:, :, : len(idx)] = hT16[:, :, idx] if not DENSE else hT16
        ccp = np.zeros(n_tok, np.float32)
        ccp[: len(wtok)] = wtok
        cc_e = np.ascontiguousarray(np.broadcast_to(ccp, (128, n_tok)))

        wu_e = w_up[e].astype(BF16).reshape(ND, 128, F).transpose(1, 0, 2)
        wg_e = w_gate[e].astype(BF16).reshape(ND, 128, F).transpose(1, 0, 2)
        wd_e = (w_down[e].astype(BF16).reshape(NF, 128, ND, 128)
                .transpose(2, 1, 0, 3))                        # [po, fi, fo, pi]
        in_maps.append({
            "ht": ht_e,
            "wu": np.ascontiguousarray(wu_e),
            "wg": np.ascontiguousarray(wg_e),
            "wd": np.ascontiguousarray(wd_e),
            "cc": cc_e,
        })

    res = run_bass_kernel_spmd(
        nc, in_maps, core_ids=list(range(E)),
        trace=TRACE, trace_cores=TRACE_CORES,
    )
    LAST["results"] = res

    out = np.zeros((T, D), np.float32)
    for e in range(E):
        o = res.results[e]["out"]                              # [128, ND, n_tok] f32
        o = o.transpose(2, 1, 0).reshape(n_tok, D)             # [tok, D]
        idx = idx_list[e]
        if DENSE:
            out += o
        else:
            out[idx] += o[: len(idx)]

    return out.reshape(B, S, D), logits


# revision 6
# speedup vs baseline: 1.0988x; 1.0988x over previous
"""Mixtral sparse-MoE block on 8 TRN2 NeuronCores, expert-parallel.

Strategy (per sharding hint): the E=8 experts are sharded 1:1 across the 8
cores; the router is evaluated on host (tiny: [8192,1024]x[1024,8]) and each
core runs its expert's SwiGLU MLP on the tokens routed to it (top-2 routing,
capacity C per expert), scaling by the renormalized routing weight on-device.
The host scatter-adds the per-expert partials (the "psum of the combine").

Device math is bf16 matmul with fp32 PSUM accumulation; per-token expert
outputs are bitwise-independent of the batch composition, so gathered
(sparse) execution matches dense-then-mask execution numerically.
"""

import os
import sys

import numpy as np
import ml_dtypes

for _p in ("/opt/trn_rl_repo", "/root/.axon_site/_ro/trn_rl_repo"):
    if os.path.isdir(_p) and _p not in sys.path:
        sys.path.append(_p)

import concourse.mybir as mybir
import concourse.tile as tile
from concourse import bacc
from concourse.bass import ds, ts
from concourse.bass_utils import run_bass_kernel_spmd

BF16 = ml_dtypes.bfloat16

B, S, D, F, E = 2, 4096, 1024, 3584, 8
T = B * S
TOK = 512            # token tile (one PSUM bank of fp32)
ND = D // 128        # 8 contraction chunks for up/gate
NF = F // 128        # 28 contraction chunks for down
CAP = 2560           # per-expert token capacity (max count ~2182 at 8.3% over-mean)
DENSE = False        # True: every core runs all T tokens (no gather)

TRACE = False        # set by test.py to collect an NTFF profile
TRACE_CORES = None
LAST = {}            # test.py reads LAST["results"] (BassKernelResults)

_CACHE = {}


WGRP = 4                 # f-chunks per up/gate weight preload DMA group


def _build(n_tok):
    """One-core Bass program: SwiGLU expert MLP over n_tok gathered tokens.

    Inputs (per core): ht [128,ND,n_tok] bf16  = h^T (D-major partition tiles)
                       wu/wg [128,ND,F] bf16   = w_up/w_gate (D on partitions)
                       wd [ND,128,NF,128] bf16 = w_down as [d_chunk][f_in][f_chunk][d_in]
                       cc [128,n_tok] f32      = routing weight per token, row-broadcast
    Output: out [128,ND,n_tok] f32 = (weighted expert output)^T
    """
    assert n_tok % 128 == 0
    nc = bacc.Bacc("TRN2", target_bir_lowering=False, debug=False, num_devices=E)
    f32, bf16 = mybir.dt.float32, mybir.dt.bfloat16
    ht_d = nc.dram_tensor("ht", [128, ND, n_tok], bf16, kind="ExternalInput").ap()
    wu_d = nc.dram_tensor("wu", [128, ND, F], bf16, kind="ExternalInput").ap()
    wg_d = nc.dram_tensor("wg", [128, ND, F], bf16, kind="ExternalInput").ap()
    wd_d = nc.dram_tensor("wd", [ND, 128, NF, 128], bf16, kind="ExternalInput").ap()
    cc_d = nc.dram_tensor("cc", [128, n_tok], f32, kind="ExternalInput").ap()
    out_d = nc.dram_tensor("out", [128, ND, n_tok], f32, kind="ExternalOutput").ap()

    tiles = [TOK] * (n_tok // TOK)
    if n_tok % TOK:
        tiles.append(n_tok % TOK)
    ngrp = NF // WGRP
    with tile.TileContext(nc) as tc:
        with (
            tc.tile_pool(name="wconst", bufs=1) as wpool,
            tc.tile_pool(name="htp", bufs=2) as htpool,
            tc.tile_pool(name="wdp", bufs=2) as wdpool,
            tc.tile_pool(name="actp", bufs=1) as actpool,
            tc.tile_pool(name="silp", bufs=2) as silpool,
            tc.tile_pool(name="outp", bufs=3) as outpool,
            tc.tile_pool(name="ps", bufs=2, space="PSUM") as pspool,
        ):
            # Preload up/gate weights in interleaved F-groups so the first
            # matmuls only wait on ~2MB, not the full 14.7MB.
            wu_g, wg_g = [], []
            for g in range(ngrp):
                wut = wpool.tile([128, ND, WGRP * 128], bf16, tag=f"wu{g}")
                nc.sync.dma_start(wut, wu_d[:, :, ts(g, WGRP * 128)])
                wu_g.append(wut)
                wgt = wpool.tile([128, ND, WGRP * 128], bf16, tag=f"wg{g}")
                nc.sync.dma_start(wgt, wg_d[:, :, ts(g, WGRP * 128)])
                wg_g.append(wgt)
            cc = wpool.tile([128, n_tok], f32, tag="cc")
            nc.sync.dma_start(cc, cc_d)

            off = 0
            for t, tn in enumerate(tiles):
                ht_t = htpool.tile([128, ND, TOK], bf16, tag="ht")
                ht = ht_t[:, :, :tn]
                nc.sync.dma_start(ht, ht_d[:, :, ds(off, tn)])
                act_t = actpool.tile([128, NF, TOK], bf16, tag="act")
                act = act_t[:, :, :tn]
                for f in range(NF):
                    wuf = wu_g[f // WGRP][:, :, ts(f % WGRP, 128)]
                    wgf = wg_g[f // WGRP][:, :, ts(f % WGRP, 128)]
                    up_t = pspool.tile([128, TOK], f32, tag="up")
                    up_ps = up_t[:, :tn]
                    gp_t = pspool.tile([128, TOK], f32, tag="gp")
                    gp_ps = gp_t[:, :tn]
                    for d in range(ND):
                        nc.tensor.matmul(up_ps, wuf[:, d, :], ht[:, d, :],
                                         start=(d == 0), stop=(d == ND - 1))
                    for d in range(ND):
                        nc.tensor.matmul(gp_ps, wgf[:, d, :], ht[:, d, :],
                                         start=(d == 0), stop=(d == ND - 1))
                    sil_t = silpool.tile([128, TOK], f32, tag="sil")
                    sil = sil_t[:, :tn]
                    nc.scalar.activation(sil, up_ps,
                                         mybir.ActivationFunctionType.Silu)
                    nc.vector.tensor_mul(act[:, f, :], sil, gp_ps)
                for po in range(ND):
                    wdt = wdpool.tile([128, NF, 128], bf16, tag="wd")
                    nc.sync.dma_start(wdt, wd_d[po])
                    o_t = pspool.tile([128, TOK], f32, tag="o")
                    o_ps = o_t[:, :tn]
                    for f in range(NF):
                        nc.tensor.matmul(o_ps, wdt[:, f, :], act[:, f, :],
                                         start=(f == 0), stop=(f == NF - 1))
                    ob_t = outpool.tile([128, TOK], f32, tag="ob")
                    ob = ob_t[:, :tn]
                    nc.vector.tensor_mul(ob, o_ps, cc[:, ds(off, tn)])
                    nc.sync.dma_start(out_d[:, po, ds(off, tn)], ob)
                off += tn
    nc.compile()
    return nc


def _get_nc(n_tok):
    if n_tok not in _CACHE:
        _CACHE[n_tok] = _build(n_tok)
    return _CACHE[n_tok]


def kernel(hidden_states, w_router, w_up, w_gate, w_down):
    h = np.asarray(hidden_states, np.float32).reshape(T, D)
    wr = np.asarray(w_router, np.float32)
    w_up = np.asarray(w_up, np.float32)
    w_gate = np.asarray(w_gate, np.float32)
    w_down = np.asarray(w_down, np.float32)

    # Router on host (fp32, same math as reference).
    logits = h @ wr                                            # [T, E]
    p = logits - logits.max(1, keepdims=True)
    np.exp(p, out=p)
    p /= p.sum(1, keepdims=True)
    sel = np.argsort(-p, axis=1, kind="stable")[:, :2]         # lax.top_k ties
    tix = np.arange(T)
    w1 = p[tix, sel[:, 0]]
    w2 = p[tix, sel[:, 1]]
    denom = w1 + w2
    rw1 = (w1 / denom).astype(np.float32)
    rw2 = (w2 / denom).astype(np.float32)

    # h^T in bf16, tiled [128, ND, T] (partition-inner on D).
    hT16 = np.ascontiguousarray(h.T).astype(BF16)              # [D, T]
    hT16 = hT16.reshape(ND, 128, T).transpose(1, 0, 2)         # [128, ND, T]

    if DENSE:
        n_tok = T
    else:
        maxcnt = int(np.bincount(sel.ravel(), minlength=E).max())
        n_tok = max(512, -(-maxcnt // 128) * 128)              # per-expert capacity
    nc = _get_nc(n_tok)

    in_maps = []
    idx_list = []
    for e in range(E):
        if DENSE:
            idx = tix
            wtok = np.where(sel[:, 0] == e, rw1,
                            np.where(sel[:, 1] == e, rw2, 0.0)).astype(np.float32)
        else:
            m = (sel[:, 0] == e) | (sel[:, 1] == e)
            idx = np.nonzero(m)[0]
            wtok = np.where(sel[:, 0] == e, rw1, rw2)[idx].astype(np.float32)
        idx_list.append(idx)

        ht_e = np.zeros((128, ND, n_tok), BF16)
        ht_e[:, :, : len(idx)] = hT16 if DENSE else hT16[:, :, idx]
        ccp = np.zeros(n_tok, np.float32)
        ccp[: len(wtok)] = wtok
        cc_e = np.ascontiguousarray(np.broadcast_to(ccp, (128, n_tok)))

        wu_e = w_up[e].astype(BF16).reshape(ND, 128, F).transpose(1, 0, 2)
        wg_e = w_gate[e].astype(BF16).reshape(ND, 128, F).transpose(1, 0, 2)
        wd_e = (w_down[e].astype(BF16).reshape(NF, 128, ND, 128)
                .transpose(2, 1, 0, 3))                        # [po, fi, fo, pi]
        in_maps.append({
            "ht": ht_e,
            "wu": np.ascontiguousarray(wu_e),
            "wg": np.ascontiguousarray(wg_e),
            "wd": np.ascontiguousarray(wd_e),
            "cc": cc_e,
        })

    res = run_bass_kernel_spmd(
        nc, in_maps, core_ids=list(range(E)),
        trace=TRACE, trace_cores=TRACE_CORES,
    )
    LAST["results"] = res

    out = np.zeros((T, D), np.float32)
    for e in range(E):
        o = res.results[e]["out"]                              # [128, ND, n_tok] f32
        o = o.transpose(2, 1, 0).reshape(n_tok, D)             # [tok, D]
        idx = idx_list[e]
        if DENSE:
            out += o
        else:
            out[idx] += o[: len(idx)]

    return out.reshape(B, S, D), logits


# revision 7
# speedup vs baseline: 1.1528x; 1.0492x over previous
"""Mixtral sparse-MoE block on 8 TRN2 NeuronCores, expert-parallel.

Strategy (per sharding hint): the E=8 experts are sharded 1:1 across the 8
cores; the router is evaluated on host (tiny: [8192,1024]x[1024,8]) and each
core runs its expert's SwiGLU MLP on the tokens routed to it (top-2 routing,
capacity C per expert), scaling by the renormalized routing weight on-device.
The host scatter-adds the per-expert partials (the "psum of the combine").

Device math is bf16 matmul with fp32 PSUM accumulation; per-token expert
outputs are bitwise-independent of the batch composition, so gathered
(sparse) execution matches dense-then-mask execution numerically.
"""

import os
import sys

import numpy as np
import ml_dtypes

for _p in ("/opt/trn_rl_repo", "/root/.axon_site/_ro/trn_rl_repo"):
    if os.path.isdir(_p) and _p not in sys.path:
        sys.path.append(_p)

import concourse.mybir as mybir
import concourse.tile as tile
from concourse import bacc
from concourse.bass import ds, ts
from concourse.bass_utils import run_bass_kernel_spmd

BF16 = ml_dtypes.bfloat16

B, S, D, F, E = 2, 4096, 1024, 3584, 8
T = B * S
TOK = 512            # token tile (one PSUM bank of fp32)
ND = D // 128        # 8 contraction chunks for up/gate
NF = F // 128        # 28 contraction chunks for down
CAP = 2560           # per-expert token capacity (max count ~2182 at 8.3% over-mean)
DENSE = False        # True: every core runs all T tokens (no gather)

TRACE = False        # set by test.py to collect an NTFF profile
TRACE_CORES = None
LAST = {}            # test.py reads LAST["results"] (BassKernelResults)

_CACHE = {}


WGRP = 4                 # f-chunks per up/gate weight preload DMA group


def _build(n_tok):
    """One-core Bass program: SwiGLU expert MLP over n_tok gathered tokens.

    Inputs (per core): ht [128,ND,n_tok] bf16  = h^T (D-major partition tiles)
                       wu/wg [128,ND,F] bf16   = w_up/w_gate (D on partitions)
                       wd [ND,128,NF,128] bf16 = w_down as [d_chunk][f_in][f_chunk][d_in]
                       cc [128,n_tok] f32      = routing weight per token, row-broadcast
    Output: out [128,ND,n_tok] f32 = (weighted expert output)^T
    """
    assert n_tok % 128 == 0
    nc = bacc.Bacc("TRN2", target_bir_lowering=False, debug=False, num_devices=E)
    f32, bf16 = mybir.dt.float32, mybir.dt.bfloat16
    ht_d = nc.dram_tensor("ht", [128, ND, n_tok], bf16, kind="ExternalInput").ap()
    wu_d = nc.dram_tensor("wu", [128, ND, F], bf16, kind="ExternalInput").ap()
    wg_d = nc.dram_tensor("wg", [128, ND, F], bf16, kind="ExternalInput").ap()
    wd_d = nc.dram_tensor("wd", [ND, 128, NF, 128], bf16, kind="ExternalInput").ap()
    cc_d = nc.dram_tensor("cc", [128, n_tok], f32, kind="ExternalInput").ap()
    out_d = nc.dram_tensor("out", [128, ND, n_tok], f32, kind="ExternalOutput").ap()

    tiles = [TOK] * (n_tok // TOK)
    if n_tok % TOK:
        tiles.append(n_tok % TOK)
    ngrp = NF // WGRP
    with tile.TileContext(nc) as tc:
        with (
            tc.tile_pool(name="wconst", bufs=1) as wpool,
            tc.tile_pool(name="htp", bufs=2) as htpool,
            tc.tile_pool(name="wdp", bufs=2) as wdpool,
            tc.tile_pool(name="actp", bufs=1) as actpool,
            tc.tile_pool(name="silp", bufs=2) as silpool,
            tc.tile_pool(name="outp", bufs=3) as outpool,
            tc.tile_pool(name="ps", bufs=2, space="PSUM") as pspool,
        ):
            # First token tile's rhs must land before the bulk weight load,
            # then up/gate weights stream in interleaved F-groups so the
            # first matmuls only wait on ~3MB, not the full 15.9MB.
            ht0 = htpool.tile([128, ND, TOK], bf16, tag="ht")
            nc.sync.dma_start(ht0[:, :, :tiles[0]], ht_d[:, :, ds(0, tiles[0])])
            wu_g, wg_g = [], []
            for g in range(ngrp):
                wut = wpool.tile([128, ND, WGRP * 128], bf16, tag=f"wu{g}")
                nc.sync.dma_start(wut, wu_d[:, :, ts(g, WGRP * 128)])
                wu_g.append(wut)
                wgt = wpool.tile([128, ND, WGRP * 128], bf16, tag=f"wg{g}")
                nc.sync.dma_start(wgt, wg_d[:, :, ts(g, WGRP * 128)])
                wg_g.append(wgt)
            cc = wpool.tile([128, n_tok], f32, tag="cc")
            nc.sync.dma_start(cc, cc_d)

            off = 0
            for t, tn in enumerate(tiles):
                if t == 0:
                    ht = ht0[:, :, :tn]
                else:
                    ht_t = htpool.tile([128, ND, TOK], bf16, tag="ht")
                    ht = ht_t[:, :, :tn]
                    nc.sync.dma_start(ht, ht_d[:, :, ds(off, tn)])
                act_t = actpool.tile([128, NF, TOK], bf16, tag="act")
                act = act_t[:, :, :tn]
                for f in range(NF):
                    wuf = wu_g[f // WGRP][:, :, ts(f % WGRP, 128)]
                    wgf = wg_g[f // WGRP][:, :, ts(f % WGRP, 128)]
                    up_t = pspool.tile([128, TOK], f32, tag="up")
                    up_ps = up_t[:, :tn]
                    gp_t = pspool.tile([128, TOK], f32, tag="gp")
                    gp_ps = gp_t[:, :tn]
                    for d in range(ND):
                        nc.tensor.matmul(up_ps, wuf[:, d, :], ht[:, d, :],
                                         start=(d == 0), stop=(d == ND - 1))
                    for d in range(ND):
                        nc.tensor.matmul(gp_ps, wgf[:, d, :], ht[:, d, :],
                                         start=(d == 0), stop=(d == ND - 1))
                    sil_t = silpool.tile([128, TOK], f32, tag="sil")
                    sil = sil_t[:, :tn]
                    nc.scalar.activation(sil, up_ps,
                                         mybir.ActivationFunctionType.Silu)
                    nc.vector.tensor_mul(act[:, f, :], sil, gp_ps)
                for po in range(ND):
                    wdt = wdpool.tile([128, NF, 128], bf16, tag="wd")
                    nc.sync.dma_start(wdt, wd_d[po])
                    o_t = pspool.tile([128, TOK], f32, tag="o")
                    o_ps = o_t[:, :tn]
                    for f in range(NF):
                        nc.tensor.matmul(o_ps, wdt[:, f, :], act[:, f, :],
                                         start=(f == 0), stop=(f == NF - 1))
                    ob_t = outpool.tile([128, TOK], f32, tag="ob")
                    ob = ob_t[:, :tn]
                    nc.vector.tensor_mul(ob, o_ps, cc[:, ds(off, tn)])
                    nc.sync.dma_start(out_d[:, po, ds(off, tn)], ob)
                off += tn
    nc.compile()
    return nc


def _get_nc(n_tok):
    if n_tok not in _CACHE:
        _CACHE[n_tok] = _build(n_tok)
    return _CACHE[n_tok]


def kernel(hidden_states, w_router, w_up, w_gate, w_down):
    h = np.asarray(hidden_states, np.float32).reshape(T, D)
    wr = np.asarray(w_router, np.float32)
    w_up = np.asarray(w_up, np.float32)
    w_gate = np.asarray(w_gate, np.float32)
    w_down = np.asarray(w_down, np.float32)

    # Router on host (fp32, same math as reference).
    logits = h @ wr                                            # [T, E]
    p = logits - logits.max(1, keepdims=True)
    np.exp(p, out=p)
    p /= p.sum(1, keepdims=True)
    sel = np.argsort(-p, axis=1, kind="stable")[:, :2]         # lax.top_k ties
    tix = np.arange(T)
    w1 = p[tix, sel[:, 0]]
    w2 = p[tix, sel[:, 1]]
    denom = w1 + w2
    rw1 = (w1 / denom).astype(np.float32)
    rw2 = (w2 / denom).astype(np.float32)

    # h^T in bf16, tiled [128, ND, T] (partition-inner on D).
    hT16 = np.ascontiguousarray(h.T).astype(BF16)              # [D, T]
    hT16 = hT16.reshape(ND, 128, T).transpose(1, 0, 2)         # [128, ND, T]

    if DENSE:
        n_tok = T
    else:
        maxcnt = int(np.bincount(sel.ravel(), minlength=E).max())
        n_tok = max(512, -(-maxcnt // 128) * 128)              # per-expert capacity
    nc = _get_nc(n_tok)

    in_maps = []
    idx_list = []
    for e in range(E):
        if DENSE:
            idx = tix
            wtok = np.where(sel[:, 0] == e, rw1,
                            np.where(sel[:, 1] == e, rw2, 0.0)).astype(np.float32)
        else:
            m = (sel[:, 0] == e) | (sel[:, 1] == e)
            idx = np.nonzero(m)[0]
            wtok = np.where(sel[:, 0] == e, rw1, rw2)[idx].astype(np.float32)
        idx_list.append(idx)

        ht_e = np.zeros((128, ND, n_tok), BF16)
        ht_e[:, :, : len(idx)] = hT16 if DENSE else hT16[:, :, idx]
        ccp = np.zeros(n_tok, np.float32)
        ccp[: len(wtok)] = wtok
        cc_e = np.ascontiguousarray(np.broadcast_to(ccp, (128, n_tok)))

        wu_e = w_up[e].astype(BF16).reshape(ND, 128, F).transpose(1, 0, 2)
        wg_e = w_gate[e].astype(BF16).reshape(ND, 128, F).transpose(1, 0, 2)
        wd_e = (w_down[e].astype(BF16).reshape(NF, 128, ND, 128)
                .transpose(2, 1, 0, 3))                        # [po, fi, fo, pi]
        in_maps.append({
            "ht": ht_e,
            "wu": np.ascontiguousarray(wu_e),
            "wg": np.ascontiguousarray(wg_e),
            "wd": np.ascontiguousarray(wd_e),
            "cc": cc_e,
        })

    res = run_bass_kernel_spmd(
        nc, in_maps, core_ids=list(range(E)),
        trace=TRACE, trace_cores=TRACE_CORES,
    )
    LAST["results"] = res

    out = np.zeros((T, D), np.float32)
    for e in range(E):
        o = res.results[e]["out"]                              # [128, ND, n_tok] f32
        o = o.transpose(2, 1, 0).reshape(n_tok, D)             # [tok, D]
        idx = idx_list[e]
        if DENSE:
            out += o
        else:
            out[idx] += o[: len(idx)]

    return out.reshape(B, S, D), logits


# revision 8
# speedup vs baseline: 1.1857x; 1.0286x over previous
"""Mixtral sparse-MoE block on 8 TRN2 NeuronCores, expert-parallel.

Strategy (per sharding hint): the E=8 experts are sharded 1:1 across the 8
cores; the router is evaluated on host (tiny: [8192,1024]x[1024,8]) and each
core runs its expert's SwiGLU MLP on the tokens routed to it (top-2 routing,
capacity C per expert), scaling by the renormalized routing weight on-device.
The host scatter-adds the per-expert partials (the "psum of the combine").

Device math is bf16 matmul with fp32 PSUM accumulation; per-token expert
outputs are bitwise-independent of the batch composition, so gathered
(sparse) execution matches dense-then-mask execution numerically.
"""

import os
import sys

import numpy as np
import ml_dtypes

for _p in ("/opt/trn_rl_repo", "/root/.axon_site/_ro/trn_rl_repo"):
    if os.path.isdir(_p) and _p not in sys.path:
        sys.path.append(_p)

import concourse.mybir as mybir
import concourse.tile as tile
from concourse import bacc
from concourse.bass import ds, ts
from concourse.bass_utils import run_bass_kernel_spmd

BF16 = ml_dtypes.bfloat16

B, S, D, F, E = 2, 4096, 1024, 3584, 8
T = B * S
TOK = 512            # token tile (one PSUM bank of fp32)
ND = D // 128        # 8 contraction chunks for up/gate
NF = F // 128        # 28 contraction chunks for down
CAP = 2560           # per-expert token capacity (max count ~2182 at 8.3% over-mean)
DENSE = False        # True: every core runs all T tokens (no gather)

TRACE = False        # set by test.py to collect an NTFF profile
TRACE_CORES = None
LAST = {}            # test.py reads LAST["results"] (BassKernelResults)

_CACHE = {}


WGRP = 4                 # f-chunks per up/gate weight preload DMA group


def _build(n_tok):
    """One-core Bass program: SwiGLU expert MLP over n_tok gathered tokens.

    Inputs (per core): ht [128,ND,n_tok] bf16  = h^T (D-major partition tiles)
                       wu/wg [128,ND,F] bf16   = w_up/w_gate (D on partitions)
                       wd [ND,128,NF,128] bf16 = w_down as [d_chunk][f_in][f_chunk][d_in]
                       cc [128,n_tok] f32      = routing weight per token, row-broadcast
    Output: out [128,ND,n_tok] f32 = (weighted expert output)^T
    """
    assert n_tok % 128 == 0
    nc = bacc.Bacc("TRN2", target_bir_lowering=False, debug=False, num_devices=E)
    f32, bf16 = mybir.dt.float32, mybir.dt.bfloat16
    ht_d = nc.dram_tensor("ht", [128, ND, n_tok], bf16, kind="ExternalInput").ap()
    wu_d = nc.dram_tensor("wu", [128, ND, F], bf16, kind="ExternalInput").ap()
    wg_d = nc.dram_tensor("wg", [128, ND, F], bf16, kind="ExternalInput").ap()
    wd_d = nc.dram_tensor("wd", [ND, 128, NF, 128], bf16, kind="ExternalInput").ap()
    cc_d = nc.dram_tensor("cc", [128, n_tok], f32, kind="ExternalInput").ap()
    out_d = nc.dram_tensor("out", [128, ND, n_tok], f32, kind="ExternalOutput").ap()

    tiles = [TOK] * (n_tok // TOK)
    if n_tok % TOK:
        tiles.append(n_tok % TOK)
    grp_sizes = [2, 2] + [4] * ((NF - 4) // 4)   # f-chunks per preload DMA
    assert sum(grp_sizes) == NF
    with tile.TileContext(nc) as tc:
        with (
            tc.tile_pool(name="wconst", bufs=1) as wpool,
            tc.tile_pool(name="htp", bufs=2) as htpool,
            tc.tile_pool(name="wdp", bufs=3) as wdpool,
            tc.tile_pool(name="actp", bufs=1) as actpool,
            tc.tile_pool(name="silp", bufs=2) as silpool,
            tc.tile_pool(name="outp", bufs=3) as outpool,
            tc.tile_pool(name="ps", bufs=3, space="PSUM") as pspool,
            tc.tile_pool(name="pso", bufs=2, space="PSUM") as psopool,
        ):
            # First token tile's rhs must land before the bulk weight load,
            # then up/gate weights stream in interleaved F-groups so the
            # first matmuls only wait on ~3MB, not the full 15.9MB.
            ht0 = htpool.tile([128, ND, TOK], bf16, tag="ht")
            nc.sync.dma_start(ht0[:, :, :tiles[0]], ht_d[:, :, ds(0, tiles[0])])
            wu_f, wg_f = [None] * NF, [None] * NF   # per-f-chunk [128, ND, 128] views
            fbase = 0
            for g, gs in enumerate(grp_sizes):
                wut = wpool.tile([128, ND, gs * 128], bf16, tag=f"wu{g}")
                nc.sync.dma_start(wut, wu_d[:, :, ds(fbase * 128, gs * 128)])
                wgt = wpool.tile([128, ND, gs * 128], bf16, tag=f"wg{g}")
                nc.sync.dma_start(wgt, wg_d[:, :, ds(fbase * 128, gs * 128)])
                for j in range(gs):
                    wu_f[fbase + j] = wut[:, :, ts(j, 128)]
                    wg_f[fbase + j] = wgt[:, :, ts(j, 128)]
                fbase += gs
            cc = wpool.tile([128, n_tok], f32, tag="cc")
            nc.sync.dma_start(cc, cc_d)

            off = 0
            for t, tn in enumerate(tiles):
                if t == 0:
                    ht = ht0[:, :, :tn]
                else:
                    ht_t = htpool.tile([128, ND, TOK], bf16, tag="ht")
                    ht = ht_t[:, :, :tn]
                    nc.sync.dma_start(ht, ht_d[:, :, ds(off, tn)])
                act_t = actpool.tile([128, NF, TOK], bf16, tag="act")
                act = act_t[:, :, :tn]
                for f in range(NF):
                    wuf = wu_f[f]
                    wgf = wg_f[f]
                    up_t = pspool.tile([128, TOK], f32, tag="up")
                    up_ps = up_t[:, :tn]
                    gp_t = pspool.tile([128, TOK], f32, tag="gp")
                    gp_ps = gp_t[:, :tn]
                    for d in range(ND):
                        nc.tensor.matmul(up_ps, wuf[:, d, :], ht[:, d, :],
                                         start=(d == 0), stop=(d == ND - 1))
                    for d in range(ND):
                        nc.tensor.matmul(gp_ps, wgf[:, d, :], ht[:, d, :],
                                         start=(d == 0), stop=(d == ND - 1))
                    sil_t = silpool.tile([128, TOK], f32, tag="sil")
                    sil = sil_t[:, :tn]
                    nc.scalar.activation(sil, up_ps,
                                         mybir.ActivationFunctionType.Silu)
                    nc.vector.tensor_mul(act[:, f, :], sil, gp_ps)
                for po in range(ND):
                    wdt = wdpool.tile([128, NF, 128], bf16, tag="wd")
                    nc.sync.dma_start(wdt, wd_d[po])
                    o_t = psopool.tile([128, TOK], f32, tag="o")
                    o_ps = o_t[:, :tn]
                    for f in range(NF):
                        nc.tensor.matmul(o_ps, wdt[:, f, :], act[:, f, :],
                                         start=(f == 0), stop=(f == NF - 1))
                    ob_t = outpool.tile([128, TOK], f32, tag="ob")
                    ob = ob_t[:, :tn]
                    nc.vector.tensor_mul(ob, o_ps, cc[:, ds(off, tn)])
                    nc.sync.dma_start(out_d[:, po, ds(off, tn)], ob)
                off += tn
    nc.compile()
    return nc


def _get_nc(n_tok):
    if n_tok not in _CACHE:
        _CACHE[n_tok] = _build(n_tok)
    return _CACHE[n_tok]


def kernel(hidden_states, w_router, w_up, w_gate, w_down):
    h = np.asarray(hidden_states, np.float32).reshape(T, D)
    wr = np.asarray(w_router, np.float32)
    w_up = np.asarray(w_up, np.float32)
    w_gate = np.asarray(w_gate, np.float32)
    w_down = np.asarray(w_down, np.float32)

    # Router on host (fp32, same math as reference).
    logits = h @ wr                                            # [T, E]
    p = logits - logits.max(1, keepdims=True)
    np.exp(p, out=p)
    p /= p.sum(1, keepdims=True)
    sel = np.argsort(-p, axis=1, kind="stable")[:, :2]         # lax.top_k ties
    tix = np.arange(T)
    w1 = p[tix, sel[:, 0]]
    w2 = p[tix, sel[:, 1]]
    denom = w1 + w2
    rw1 = (w1 / denom).astype(np.float32)
    rw2 = (w2 / denom).astype(np.float32)

    # h^T in bf16, tiled [128, ND, T] (partition-inner on D).
    hT16 = np.ascontiguousarray(h.T).astype(BF16)              # [D, T]
    hT16 = hT16.reshape(ND, 128, T).transpose(1, 0, 2)         # [128, ND, T]

    if DENSE:
        n_tok = T
    else:
        maxcnt = int(np.bincount(sel.ravel(), minlength=E).max())
        n_tok = max(512, -(-maxcnt // 128) * 128)              # per-expert capacity
    nc = _get_nc(n_tok)

    in_maps = []
    idx_list = []
    for e in range(E):
        if DENSE:
            idx = tix
            wtok = np.where(sel[:, 0] == e, rw1,
                            np.where(sel[:, 1] == e, rw2, 0.0)).astype(np.float32)
        else:
            m = (sel[:, 0] == e) | (sel[:, 1] == e)
            idx = np.nonzero(m)[0]
            wtok = np.where(sel[:, 0] == e, rw1, rw2)[idx].astype(np.float32)
        idx_list.append(idx)

        ht_e = np.zeros((128, ND, n_tok), BF16)
        ht_e[:, :, : len(idx)] = hT16 if DENSE else hT16[:, :, idx]
        ccp = np.zeros(n_tok, np.float32)
        ccp[: len(wtok)] = wtok
        cc_e = np.ascontiguousarray(np.broadcast_to(ccp, (128, n_tok)))

        wu_e = w_up[e].astype(BF16).reshape(ND, 128, F).transpose(1, 0, 2)
        wg_e = w_gate[e].astype(BF16).reshape(ND, 128, F).transpose(1, 0, 2)
        wd_e = (w_down[e].astype(BF16).reshape(NF, 128, ND, 128)
                .transpose(2, 1, 0, 3))                        # [po, fi, fo, pi]
        in_maps.append({
            "ht": ht_e,
            "wu": np.ascontiguousarray(wu_e),
            "wg": np.ascontiguousarray(wg_e),
            "wd": np.ascontiguousarray(wd_e),
            "cc": cc_e,
        })

    res = run_bass_kernel_spmd(
        nc, in_maps, core_ids=list(range(E)),
        trace=TRACE, trace_cores=TRACE_CORES,
    )
    LAST["results"] = res

    out = np.zeros((T, D), np.float32)
    for e in range(E):
        o = res.results[e]["out"]                              # [128, ND, n_tok] f32
        o = o.transpose(2, 1, 0).reshape(n_tok, D)             # [tok, D]
        idx = idx_list[e]
        if DENSE:
            out += o
        else:
            out[idx] += o[: len(idx)]

    return out.reshape(B, S, D), logits


# revision 9
# speedup vs baseline: 1.2511x; 1.0552x over previous
"""Mixtral sparse-MoE block on 8 TRN2 NeuronCores, expert-parallel.

Strategy (per sharding hint): the E=8 experts are sharded across the 8 cores
and the router is replicated on host (tiny: [8192,1024]x[1024,8]). Each
expert's FFN dim F is split in half across a core pair, and heavy-count
experts are paired with light ones so every core gets a near-equal token
load (top-2 routing, per-slot capacity). Each core runs two phases: slot-A
expert's half-FFN over its routed tokens, then slot-B's. The host applies
the renormalized routing weights and scatter-adds the partials (the "psum
of the combine").

Device math is bf16 matmul with fp32 PSUM accumulation; per-token expert
outputs are independent of batch composition, so gathered (sparse)
execution matches dense-then-mask execution numerically.
"""

import os
import sys

import numpy as np
import ml_dtypes

for _p in ("/opt/trn_rl_repo", "/root/.axon_site/_ro/trn_rl_repo"):
    if os.path.isdir(_p) and _p not in sys.path:
        sys.path.append(_p)

import concourse.mybir as mybir
import concourse.tile as tile
from concourse import bacc
from concourse.bass import ds, ts
from concourse.bass_utils import run_bass_kernel_spmd

BF16 = ml_dtypes.bfloat16

B, S, D, F, E = 2, 4096, 1024, 3584, 8
T = B * S
TOK = 512            # token tile (one PSUM bank of fp32)
ND = D // 128        # 8 contraction chunks for up/gate
F2 = F // 2          # FFN half per core
NF2 = F2 // 128      # 14 chunks for the down contraction

TRACE = False        # set by test.py to collect an NTFF profile
TRACE_CORES = None
LAST = {}            # test.py reads LAST["results"] (BassKernelResults)

_CACHE = {}

# Weight preload DMA granularity: fine groups up front so the first matmuls
# wait on ~1.5MB, coarser after.
GRP_SIZES = [2, 2, 2] + [4] * ((NF2 - 6) // 4)
assert sum(GRP_SIZES) == NF2


def _tiles_of(n):
    t = [TOK] * (n // TOK)
    if n % TOK:
        t.append(n % TOK)
    return t


def _build(caps):
    """One-core Bass program: two half-FFN SwiGLU phases (slot A, slot B).

    Inputs: ht [128,ND,CT] bf16      = h^T of slot-A tokens then slot-B tokens
            wu/wg [2,128,ND,F2] bf16 = slot's w_up/w_gate half (D on partitions)
            wd [2,ND,128,NF2,128] bf16 = slot's w_down half
    Output: out [128,ND,CT] f32 = unscaled half-FFN expert output^T
    """
    ca, cb = caps
    ct = ca + cb
    nc = bacc.Bacc("TRN2", target_bir_lowering=False, debug=False, num_devices=E)
    f32, bf16 = mybir.dt.float32, mybir.dt.bfloat16
    ht_d = nc.dram_tensor("ht", [128, ND, ct], bf16, kind="ExternalInput").ap()
    wu_d = nc.dram_tensor("wu", [2, 128, ND, F2], bf16, kind="ExternalInput").ap()
    wg_d = nc.dram_tensor("wg", [2, 128, ND, F2], bf16, kind="ExternalInput").ap()
    wd_d = nc.dram_tensor("wd", [2, ND, 128, NF2, 128], bf16,
                          kind="ExternalInput").ap()
    out_d = nc.dram_tensor("out", [128, ND, ct], f32, kind="ExternalOutput").ap()

    fbases = np.cumsum([0] + GRP_SIZES[:-1]).tolist()

    with tile.TileContext(nc) as tc:
        with (
            tc.tile_pool(name="wconst", bufs=1) as wpool,
            tc.tile_pool(name="htp", bufs=2) as htpool,
            tc.tile_pool(name="wdp", bufs=3) as wdpool,
            tc.tile_pool(name="actp", bufs=2) as actpool,
            tc.tile_pool(name="silp", bufs=2) as silpool,
            tc.tile_pool(name="outp", bufs=3) as outpool,
            tc.tile_pool(name="ps", bufs=3, space="PSUM") as pspool,
            tc.tile_pool(name="pso", bufs=2, space="PSUM") as psopool,
        ):
            wu_f = [[None] * NF2, [None] * NF2]
            wg_f = [[None] * NF2, [None] * NF2]

            def load_wgrp(s, g):
                gs, fbase = GRP_SIZES[g], fbases[g]
                wut = wpool.tile([128, ND, gs * 128], bf16, tag=f"wu{s}_{g}")
                nc.sync.dma_start(wut, wu_d[s][:, :, ds(fbase * 128, gs * 128)])
                wgt = wpool.tile([128, ND, gs * 128], bf16, tag=f"wg{s}_{g}")
                nc.sync.dma_start(wgt, wg_d[s][:, :, ds(fbase * 128, gs * 128)])
                for j in range(gs):
                    wu_f[s][fbase + j] = wut[:, :, ts(j, 128)]
                    wg_f[s][fbase + j] = wgt[:, :, ts(j, 128)]

            # Phase-A token tile 0 rhs first, then slot-A weights.
            tiles_a, tiles_b = _tiles_of(ca), _tiles_of(cb)
            ht0 = htpool.tile([128, ND, TOK], bf16, tag="ht")
            nc.sync.dma_start(ht0[:, :, :tiles_a[0]], ht_d[:, :, ds(0, tiles_a[0])])
            for g in range(len(GRP_SIZES)):
                load_wgrp(0, g)

            # Slot-B weight groups are emitted interleaved with phase A's tile
            # loop so they never queue ahead of phase-A token DMAs.
            bgroups = list(range(len(GRP_SIZES)))

            off = 0
            for s, tiles in ((0, tiles_a), (1, tiles_b)):
                for t, tn in enumerate(tiles):
                    if s == 0 and t == 0:
                        ht = ht0[:, :, :tn]
                    else:
                        ht_t = htpool.tile([128, ND, TOK], bf16, tag="ht")
                        ht = ht_t[:, :, :tn]
                        nc.sync.dma_start(ht, ht_d[:, :, ds(off, tn)])
                    act_t = actpool.tile([128, NF2, TOK], bf16, tag="act")
                    act = act_t[:, :, :tn]
                    for f in range(NF2):
                        up_t = pspool.tile([128, TOK], f32, tag="up")
                        up_ps = up_t[:, :tn]
                        gp_t = pspool.tile([128, TOK], f32, tag="gp")
                        gp_ps = gp_t[:, :tn]
                        for d in range(ND):
                            nc.tensor.matmul(up_ps, wu_f[s][f][:, d, :], ht[:, d, :],
                                             start=(d == 0), stop=(d == ND - 1))
                        for d in range(ND):
                            nc.tensor.matmul(gp_ps, wg_f[s][f][:, d, :], ht[:, d, :],
                                             start=(d == 0), stop=(d == ND - 1))
                        sil_t = silpool.tile([128, TOK], f32, tag="sil")
                        sil = sil_t[:, :tn]
                        nc.scalar.activation(sil, up_ps,
                                             mybir.ActivationFunctionType.Silu)
                        nc.vector.tensor_mul(act[:, f, :], sil, gp_ps)
                    for po in range(ND):
                        wdt = wdpool.tile([128, NF2, 128], bf16, tag="wd")
                        nc.sync.dma_start(wdt, wd_d[s, po])
                        o_t = psopool.tile([128, TOK], f32, tag="o")
                        o_ps = o_t[:, :tn]
                        for f in range(NF2):
                            nc.tensor.matmul(o_ps, wdt[:, f, :], act[:, f, :],
                                             start=(f == 0), stop=(f == NF2 - 1))
                        ob_t = outpool.tile([128, TOK], f32, tag="ob")
                        ob = ob_t[:, :tn]
                        nc.vector.tensor_copy(ob, o_ps)
                        nc.sync.dma_start(out_d[:, po, ds(off, tn)], ob)
                    # Trickle in slot-B weights during phase A.
                    if s == 0 and t > 0 and bgroups:
                        n_emit = -(-len(GRP_SIZES) // max(1, len(tiles_a) - 1))
                        for _ in range(n_emit):
                            if bgroups:
                                load_wgrp(1, bgroups.pop(0))
                    off += tn
                # Ensure all slot-B groups are loaded even for tiny phase A.
                if s == 0:
                    while bgroups:
                        load_wgrp(1, bgroups.pop(0))
    nc.compile()
    return nc


def _get_nc(caps):
    if caps not in _CACHE:
        _CACHE[caps] = _build(caps)
    return _CACHE[caps]


def _cap(n):
    return max(512, -(-n // 128) * 128)


def kernel(hidden_states, w_router, w_up, w_gate, w_down):
    h = np.asarray(hidden_states, np.float32).reshape(T, D)
    wr = np.asarray(w_router, np.float32)
    w_up = np.asarray(w_up, np.float32)
    w_gate = np.asarray(w_gate, np.float32)
    w_down = np.asarray(w_down, np.float32)

    # Router on host (fp32, same math as reference).
    logits = h @ wr                                            # [T, E]
    p = logits - logits.max(1, keepdims=True)
    np.exp(p, out=p)
    p /= p.sum(1, keepdims=True)
    sel = np.argsort(-p, axis=1, kind="stable")[:, :2]         # lax.top_k ties
    tix = np.arange(T)
    w1 = p[tix, sel[:, 0]]
    w2 = p[tix, sel[:, 1]]
    denom = w1 + w2
    rw1 = (w1 / denom).astype(np.float32)
    rw2 = (w2 / denom).astype(np.float32)

    # Token list + routing weight per expert.
    idx_e, wt_e = [], []
    for e in range(E):
        m = (sel[:, 0] == e) | (sel[:, 1] == e)
        idx = np.nonzero(m)[0]
        idx_e.append(idx)
        wt_e.append(np.where(sel[:, 0] == e, rw1, rw2)[idx].astype(np.float32))
    counts = np.array([len(i) for i in idx_e])

    # Pair heavy experts with light ones; each pair spans 2 cores (F halves).
    order = np.argsort(-counts, kind="stable")
    big, small = order[:4], order[4:][::-1]
    ca = _cap(int(counts[big].max()))
    cb = _cap(int(counts[small].max()))
    nc = _get_nc((ca, cb))

    # h^T in bf16, tiled [128, ND, T] (partition-inner on D).
    hT16 = np.ascontiguousarray(h.T).astype(BF16)              # [D, T]
    hT16 = hT16.reshape(ND, 128, T).transpose(1, 0, 2)         # [128, ND, T]

    def wslice(w, e, half):                                    # [D, F2] -> tiled
        ww = w[e][:, half * F2:(half + 1) * F2].astype(BF16)
        return ww.reshape(ND, 128, F2).transpose(1, 0, 2)      # [128, ND, F2]

    def wdslice(e, half):                                      # [F2, D] -> tiled
        ww = w_down[e][half * F2:(half + 1) * F2].astype(BF16)
        return ww.reshape(NF2, 128, ND, 128).transpose(2, 1, 0, 3)

    in_maps = []
    for pair in range(4):
        ea, eb = int(big[pair]), int(small[pair])
        ht_p = np.zeros((128, ND, ca + cb), BF16)
        ht_p[:, :, :counts[ea]] = hT16[:, :, idx_e[ea]]
        ht_p[:, :, ca:ca + counts[eb]] = hT16[:, :, idx_e[eb]]
        for half in range(2):
            in_maps.append({
                "ht": ht_p,
                "wu": np.ascontiguousarray(
                    np.stack([wslice(w_up, ea, half), wslice(w_up, eb, half)])),
                "wg": np.ascontiguousarray(
                    np.stack([wslice(w_gate, ea, half), wslice(w_gate, eb, half)])),
                "wd": np.ascontiguousarray(
                    np.stack([wdslice(ea, half), wdslice(eb, half)])),
            })

    res = run_bass_kernel_spmd(
        nc, in_maps, core_ids=list(range(E)),
        trace=TRACE, trace_cores=TRACE_CORES,
    )
    LAST["results"] = res

    out = np.zeros((T, D), np.float32)
    for pair in range(4):
        ea, eb = int(big[pair]), int(small[pair])
        oa = res.results[2 * pair]["out"]                      # [128, ND, ca+cb]
        obb = res.results[2 * pair + 1]["out"]
        o = (oa + obb).transpose(2, 1, 0).reshape(ca + cb, D)  # [tok, D]
        out[idx_e[ea]] += o[:counts[ea]] * wt_e[ea][:, None]
        out[idx_e[eb]] += o[ca:ca + counts[eb]] * wt_e[eb][:, None]

    return out.reshape(B, S, D), logits


# revision 10
# speedup vs baseline: 1.2647x; 1.0108x over previous
"""Mixtral sparse-MoE block on 8 TRN2 NeuronCores, expert-parallel.

Strategy (per sharding hint): the E=8 experts are sharded across the 8 cores
and the router is replicated on host (tiny: [8192,1024]x[1024,8]). Each
expert's FFN dim F is split in half across a core pair, and heavy-count
experts are paired with light ones so every core gets a near-equal token
load (top-2 routing, per-slot capacity). Each core runs two phases: slot-A
expert's half-FFN over its routed tokens, then slot-B's. The host applies
the renormalized routing weights and scatter-adds the partials (the "psum
of the combine").

Device math is bf16 matmul with fp32 PSUM accumulation; per-token expert
outputs are independent of batch composition, so gathered (sparse)
execution matches dense-then-mask execution numerically.
"""

import os
import sys

import numpy as np
import ml_dtypes

for _p in ("/opt/trn_rl_repo", "/root/.axon_site/_ro/trn_rl_repo"):
    if os.path.isdir(_p) and _p not in sys.path:
        sys.path.append(_p)

import concourse.mybir as mybir
import concourse.tile as tile
from concourse import bacc
from concourse.bass import ds, ts
from concourse.bass_utils import run_bass_kernel_spmd

BF16 = ml_dtypes.bfloat16

B, S, D, F, E = 2, 4096, 1024, 3584, 8
T = B * S
TOK = 512            # token tile (one PSUM bank of fp32)
ND = D // 128        # 8 contraction chunks for up/gate
F2 = F // 2          # FFN half per core
NF2 = F2 // 128      # 14 chunks for the down contraction

TRACE = False        # set by test.py to collect an NTFF profile
TRACE_CORES = None
LAST = {}            # test.py reads LAST["results"] (BassKernelResults)

_CACHE = {}

# Weight preload DMA granularity: fine groups up front so the first matmuls
# wait on ~1.5MB, coarser after.
GRP_SIZES = [2, 2, 2] + [4] * ((NF2 - 6) // 4)
assert sum(GRP_SIZES) == NF2


def _tiles_of(n):
    t = [TOK] * (n // TOK)
    if n % TOK:
        t.append(n % TOK)
    return t


def _build(caps):
    """One-core Bass program: two half-FFN SwiGLU phases (slot A, slot B).

    Inputs: ht [128,ND,CT] bf16      = h^T of slot-A tokens then slot-B tokens
            wu/wg [2,128,ND,F2] bf16 = slot's w_up/w_gate half (D on partitions)
            wd [2,ND,128,NF2,128] bf16 = slot's w_down half
    Output: out [128,ND,CT] f32 = unscaled half-FFN expert output^T
    """
    ca, cb = caps
    ct = ca + cb
    nc = bacc.Bacc("TRN2", target_bir_lowering=False, debug=False, num_devices=E)
    f32, bf16 = mybir.dt.float32, mybir.dt.bfloat16
    ht_d = nc.dram_tensor("ht", [128, ND, ct], bf16, kind="ExternalInput").ap()
    wu_d = nc.dram_tensor("wu", [2, 128, ND, F2], bf16, kind="ExternalInput").ap()
    wg_d = nc.dram_tensor("wg", [2, 128, ND, F2], bf16, kind="ExternalInput").ap()
    wd_d = nc.dram_tensor("wd", [2, ND, 128, NF2, 128], bf16,
                          kind="ExternalInput").ap()
    out_d = nc.dram_tensor("out", [128, ND, ct], f32, kind="ExternalOutput").ap()

    fbases = np.cumsum([0] + GRP_SIZES[:-1]).tolist()

    with tile.TileContext(nc) as tc:
        with (
            tc.tile_pool(name="wconst", bufs=1) as wpool,
            tc.tile_pool(name="htp", bufs=2) as htpool,
            tc.tile_pool(name="wdp", bufs=3) as wdpool,
            tc.tile_pool(name="actp", bufs=2) as actpool,
            tc.tile_pool(name="silp", bufs=2) as silpool,
            tc.tile_pool(name="outp", bufs=3) as outpool,
            tc.tile_pool(name="ps", bufs=3, space="PSUM") as pspool,
            tc.tile_pool(name="pso", bufs=2, space="PSUM") as psopool,
        ):
            wu_f = [[None] * NF2, [None] * NF2]
            wg_f = [[None] * NF2, [None] * NF2]

            def load_wgrp(s, g):
                gs, fbase = GRP_SIZES[g], fbases[g]
                wut = wpool.tile([128, ND, gs * 128], bf16, tag=f"wu{s}_{g}")
                nc.sync.dma_start(wut, wu_d[s][:, :, ds(fbase * 128, gs * 128)])
                wgt = wpool.tile([128, ND, gs * 128], bf16, tag=f"wg{s}_{g}")
                nc.sync.dma_start(wgt, wg_d[s][:, :, ds(fbase * 128, gs * 128)])
                for j in range(gs):
                    wu_f[s][fbase + j] = wut[:, :, ts(j, 128)]
                    wg_f[s][fbase + j] = wgt[:, :, ts(j, 128)]

            # Phase-A token tile 0 rhs first, then slot-A weights.
            tiles_a, tiles_b = _tiles_of(ca), _tiles_of(cb)
            ht0 = htpool.tile([128, ND, TOK], bf16, tag="ht")
            nc.sync.dma_start(ht0[:, :, :tiles_a[0]], ht_d[:, :, ds(0, tiles_a[0])])
            for g in range(len(GRP_SIZES)):
                load_wgrp(0, g)

            # Slot-B weight groups are emitted interleaved with phase A's tile
            # loop so they never queue ahead of phase-A token DMAs.
            bgroups = list(range(len(GRP_SIZES)))
            htb0 = htpool.tile([128, ND, TOK], bf16, tag="htb0")
            htb0_loaded = [False]

            off = 0
            for s, tiles in ((0, tiles_a), (1, tiles_b)):
                for t, tn in enumerate(tiles):
                    if s == 0 and t == 0:
                        ht = ht0[:, :, :tn]
                    elif s == 1 and t == 0:
                        ht = htb0[:, :, :tn]
                        if not htb0_loaded[0]:
                            nc.sync.dma_start(ht, ht_d[:, :, ds(off, tn)])
                    else:
                        ht_t = htpool.tile([128, ND, TOK], bf16, tag="ht")
                        ht = ht_t[:, :, :tn]
                        nc.sync.dma_start(ht, ht_d[:, :, ds(off, tn)])
                    act_t = actpool.tile([128, NF2, TOK], bf16, tag="act")
                    act = act_t[:, :, :tn]
                    for f in range(NF2):
                        up_t = pspool.tile([128, TOK], f32, tag="up")
                        up_ps = up_t[:, :tn]
                        gp_t = pspool.tile([128, TOK], f32, tag="gp")
                        gp_ps = gp_t[:, :tn]
                        for d in range(ND):
                            nc.tensor.matmul(up_ps, wu_f[s][f][:, d, :], ht[:, d, :],
                                             start=(d == 0), stop=(d == ND - 1))
                        for d in range(ND):
                            nc.tensor.matmul(gp_ps, wg_f[s][f][:, d, :], ht[:, d, :],
                                             start=(d == 0), stop=(d == ND - 1))
                        sil_t = silpool.tile([128, TOK], f32, tag="sil")
                        sil = sil_t[:, :tn]
                        nc.scalar.activation(sil, up_ps,
                                             mybir.ActivationFunctionType.Silu)
                        nc.vector.tensor_mul(act[:, f, :], sil, gp_ps)
                    for po in range(ND):
                        wdt = wdpool.tile([128, NF2, 128], bf16, tag="wd")
                        nc.sync.dma_start(wdt, wd_d[s, po])
                        o_t = psopool.tile([128, TOK], f32, tag="o")
                        o_ps = o_t[:, :tn]
                        for f in range(NF2):
                            nc.tensor.matmul(o_ps, wdt[:, f, :], act[:, f, :],
                                             start=(f == 0), stop=(f == NF2 - 1))
                        ob_t = outpool.tile([128, TOK], f32, tag="ob")
                        ob = ob_t[:, :tn]
                        nc.vector.tensor_copy(ob, o_ps)
                        nc.sync.dma_start(out_d[:, po, ds(off, tn)], ob)
                    # Trickle in slot-B weights during phase A.
                    if s == 0 and t > 0 and bgroups:
                        n_emit = -(-len(GRP_SIZES) // max(1, len(tiles_a) - 1))
                        for _ in range(n_emit):
                            if bgroups:
                                load_wgrp(1, bgroups.pop(0))
                        if not bgroups and not htb0_loaded[0]:
                            nc.sync.dma_start(
                                htb0[:, :, :tiles_b[0]],
                                ht_d[:, :, ds(ca, tiles_b[0])])
                            htb0_loaded[0] = True
                    off += tn
                # Ensure all slot-B groups are loaded even for tiny phase A.
                if s == 0:
                    while bgroups:
                        load_wgrp(1, bgroups.pop(0))
    nc.compile()
    return nc


def _get_nc(caps):
    if caps not in _CACHE:
        _CACHE[caps] = _build(caps)
    return _CACHE[caps]


def _cap(n):
    return max(512, -(-n // 16) * 16)


def kernel(hidden_states, w_router, w_up, w_gate, w_down):
    h = np.asarray(hidden_states, np.float32).reshape(T, D)
    wr = np.asarray(w_router, np.float32)
    w_up = np.asarray(w_up, np.float32)
    w_gate = np.asarray(w_gate, np.float32)
    w_down = np.asarray(w_down, np.float32)

    # Router on host (fp32, same math as reference).
    logits = h @ wr                                            # [T, E]
    p = logits - logits.max(1, keepdims=True)
    np.exp(p, out=p)
    p /= p.sum(1, keepdims=True)
    sel = np.argsort(-p, axis=1, kind="stable")[:, :2]         # lax.top_k ties
    tix = np.arange(T)
    w1 = p[tix, sel[:, 0]]
    w2 = p[tix, sel[:, 1]]
    denom = w1 + w2
    rw1 = (w1 / denom).astype(np.float32)
    rw2 = (w2 / denom).astype(np.float32)

    # Token list + routing weight per expert.
    idx_e, wt_e = [], []
    for e in range(E):
        m = (sel[:, 0] == e) | (sel[:, 1] == e)
        idx = np.nonzero(m)[0]
        idx_e.append(idx)
        wt_e.append(np.where(sel[:, 0] == e, rw1, rw2)[idx].astype(np.float32))
    counts = np.array([len(i) for i in idx_e])

    # Pair heavy experts with light ones; each pair spans 2 cores (F halves).
    order = np.argsort(-counts, kind="stable")
    big, small = order[:4], order[4:][::-1]
    ca = _cap(int(counts[big].max()))
    cb = _cap(int(counts[small].max()))
    nc = _get_nc((ca, cb))

    # h^T in bf16, tiled [128, ND, T] (partition-inner on D).
    hT16 = np.ascontiguousarray(h.T).astype(BF16)              # [D, T]
    hT16 = hT16.reshape(ND, 128, T).transpose(1, 0, 2)         # [128, ND, T]

    def wslice(w, e, half):                                    # [D, F2] -> tiled
        ww = w[e][:, half * F2:(half + 1) * F2].astype(BF16)
        return ww.reshape(ND, 128, F2).transpose(1, 0, 2)      # [128, ND, F2]

    def wdslice(e, half):                                      # [F2, D] -> tiled
        ww = w_down[e][half * F2:(half + 1) * F2].astype(BF16)
        return ww.reshape(NF2, 128, ND, 128).transpose(2, 1, 0, 3)

    in_maps = []
    for pair in range(4):
        ea, eb = int(big[pair]), int(small[pair])
        ht_p = np.zeros((128, ND, ca + cb), BF16)
        ht_p[:, :, :counts[ea]] = hT16[:, :, idx_e[ea]]
        ht_p[:, :, ca:ca + counts[eb]] = hT16[:, :, idx_e[eb]]
        for half in range(2):
            in_maps.append({
                "ht": ht_p,
                "wu": np.ascontiguousarray(
                    np.stack([wslice(w_up, ea, half), wslice(w_up, eb, half)])),
                "wg": np.ascontiguousarray(
                    np.stack([wslice(w_gate, ea, half), wslice(w_gate, eb, half)])),
                "wd": np.ascontiguousarray(
                    np.stack([wdslice(ea, half), wdslice(eb, half)])),
            })

    res = run_bass_kernel_spmd(
        nc, in_maps, core_ids=list(range(E)),
        trace=TRACE, trace_cores=TRACE_CORES,
    )
    LAST["results"] = res

    out = np.zeros((T, D), np.float32)
    for pair in range(4):
        ea, eb = int(big[pair]), int(small[pair])
        oa = res.results[2 * pair]["out"]                      # [128, ND, ca+cb]
        obb = res.results[2 * pair + 1]["out"]
        o = (oa + obb).transpose(2, 1, 0).reshape(ca + cb, D)  # [tok, D]
        out[idx_e[ea]] += o[:counts[ea]] * wt_e[ea][:, None]
        out[idx_e[eb]] += o[ca:ca + counts[eb]] * wt_e[eb][:, None]

    return out.reshape(B, S, D), logits


# revision 11
# speedup vs baseline: 1.2661x; 1.0011x over previous
"""Mixtral sparse-MoE block on 8 TRN2 NeuronCores, expert-parallel.

Strategy (per sharding hint): the E=8 experts are sharded across the 8 cores
and the router is replicated on host (tiny: [8192,1024]x[1024,8]). Each
expert's FFN dim F is split in half across a core pair, and heavy-count
experts are paired with light ones so every core gets a near-equal token
load (top-2 routing, per-slot capacity). Each core runs two phases: slot-A
expert's half-FFN over its routed tokens, then slot-B's. The host applies
the renormalized routing weights and scatter-adds the partials (the "psum
of the combine").

Device math is bf16 matmul with fp32 PSUM accumulation; per-token expert
outputs are independent of batch composition, so gathered (sparse)
execution matches dense-then-mask execution numerically.
"""

import os
import sys

import numpy as np
import ml_dtypes

for _p in ("/opt/trn_rl_repo", "/root/.axon_site/_ro/trn_rl_repo"):
    if os.path.isdir(_p) and _p not in sys.path:
        sys.path.append(_p)

import concourse.mybir as mybir
import concourse.tile as tile
from concourse import bacc
from concourse.bass import ds, ts
from concourse.bass_utils import run_bass_kernel_spmd

BF16 = ml_dtypes.bfloat16

B, S, D, F, E = 2, 4096, 1024, 3584, 8
T = B * S
TOK = 512            # token tile (one PSUM bank of fp32)
ND = D // 128        # 8 contraction chunks for up/gate
F2 = F // 2          # FFN half per core
NF2 = F2 // 128      # 14 chunks for the down contraction

TRACE = False        # set by test.py to collect an NTFF profile
TRACE_CORES = None
LAST = {}            # test.py reads LAST["results"] (BassKernelResults)

_CACHE = {}

# Weight preload DMA granularity: fine groups up front so the first matmuls
# wait on ~1.5MB, coarser after.
GRP_SIZES = [2, 2, 2] + [4] * ((NF2 - 6) // 4)
assert sum(GRP_SIZES) == NF2


def _tiles_of(n):
    t = [TOK] * (n // TOK)
    if n % TOK:
        t.append(n % TOK)
    return t


def _build(caps):
    """One-core Bass program: two half-FFN SwiGLU phases (slot A, slot B).

    Inputs: ht [128,ND,CT] bf16      = h^T of slot-A tokens then slot-B tokens
            wu/wg [2,128,ND,F2] bf16 = slot's w_up/w_gate half (D on partitions)
            wd [2,ND,128,NF2,128] bf16 = slot's w_down half
    Output: out [128,ND,CT] f32 = unscaled half-FFN expert output^T
    """
    ca, cb = caps
    ct = ca + cb
    nc = bacc.Bacc("TRN2", target_bir_lowering=False, debug=False, num_devices=E)
    f32, bf16 = mybir.dt.float32, mybir.dt.bfloat16
    ht_d = nc.dram_tensor("ht", [128, ND, ct], bf16, kind="ExternalInput").ap()
    wu_d = nc.dram_tensor("wu", [2, 128, ND, F2], bf16, kind="ExternalInput").ap()
    wg_d = nc.dram_tensor("wg", [2, 128, ND, F2], bf16, kind="ExternalInput").ap()
    wd_d = nc.dram_tensor("wd", [2, ND, 128, NF2, 128], bf16,
                          kind="ExternalInput").ap()
    out_d = nc.dram_tensor("out", [128, ND, ct], f32, kind="ExternalOutput").ap()

    fbases = np.cumsum([0] + GRP_SIZES[:-1]).tolist()

    with tile.TileContext(nc) as tc:
        with (
            tc.tile_pool(name="wconst", bufs=1) as wpool,
            tc.tile_pool(name="htp", bufs=2) as htpool,
            tc.tile_pool(name="wdp", bufs=3) as wdpool,
            tc.tile_pool(name="actp", bufs=2) as actpool,
            tc.tile_pool(name="silp", bufs=2) as silpool,
            tc.tile_pool(name="outp", bufs=3) as outpool,
            tc.tile_pool(name="ps", bufs=3, space="PSUM") as pspool,
            tc.tile_pool(name="pso", bufs=2, space="PSUM") as psopool,
        ):
            wu_f = [[None] * NF2, [None] * NF2]
            wg_f = [[None] * NF2, [None] * NF2]

            def load_wgrp(s, g):
                gs, fbase = GRP_SIZES[g], fbases[g]
                wut = wpool.tile([128, ND, gs * 128], bf16, tag=f"wu{s}_{g}")
                nc.sync.dma_start(wut, wu_d[s][:, :, ds(fbase * 128, gs * 128)])
                wgt = wpool.tile([128, ND, gs * 128], bf16, tag=f"wg{s}_{g}")
                nc.sync.dma_start(wgt, wg_d[s][:, :, ds(fbase * 128, gs * 128)])
                for j in range(gs):
                    wu_f[s][fbase + j] = wut[:, :, ts(j, 128)]
                    wg_f[s][fbase + j] = wgt[:, :, ts(j, 128)]

            # Phase-A token tile 0 rhs first, then slot-A weights.
            tiles_a, tiles_b = _tiles_of(ca), _tiles_of(cb)
            ht0 = htpool.tile([128, ND, TOK], bf16, tag="ht")
            nc.sync.dma_start(ht0[:, :2, :tiles_a[0]], ht_d[:, :2, ds(0, tiles_a[0])])
            load_wgrp(0, 0)
            nc.sync.dma_start(ht0[:, 2:, :tiles_a[0]], ht_d[:, 2:, ds(0, tiles_a[0])])
            for g in range(1, len(GRP_SIZES)):
                load_wgrp(0, g)

            # Slot-B weight groups are emitted interleaved with phase A's tile
            # loop so they never queue ahead of phase-A token DMAs.
            bgroups = list(range(len(GRP_SIZES)))
            htb0 = htpool.tile([128, ND, TOK], bf16, tag="htb0")
            htb0_loaded = [False]

            off = 0
            for s, tiles in ((0, tiles_a), (1, tiles_b)):
                for t, tn in enumerate(tiles):
                    if s == 0 and t == 0:
                        ht = ht0[:, :, :tn]
                    elif s == 1 and t == 0:
                        ht = htb0[:, :, :tn]
                        if not htb0_loaded[0]:
                            nc.sync.dma_start(ht, ht_d[:, :, ds(off, tn)])
                    else:
                        ht_t = htpool.tile([128, ND, TOK], bf16, tag="ht")
                        ht = ht_t[:, :, :tn]
                        nc.sync.dma_start(ht, ht_d[:, :, ds(off, tn)])
                    act_t = actpool.tile([128, NF2, TOK], bf16, tag="act")
                    act = act_t[:, :, :tn]
                    for f in range(NF2):
                        up_t = pspool.tile([128, TOK], f32, tag="up")
                        up_ps = up_t[:, :tn]
                        gp_t = pspool.tile([128, TOK], f32, tag="gp")
                        gp_ps = gp_t[:, :tn]
                        for d in range(ND):
                            nc.tensor.matmul(up_ps, wu_f[s][f][:, d, :], ht[:, d, :],
                                             start=(d == 0), stop=(d == ND - 1))
                        for d in range(ND):
                            nc.tensor.matmul(gp_ps, wg_f[s][f][:, d, :], ht[:, d, :],
                                             start=(d == 0), stop=(d == ND - 1))
                        sil_t = silpool.tile([128, TOK], f32, tag="sil")
                        sil = sil_t[:, :tn]
                        nc.scalar.activation(sil, up_ps,
                                             mybir.ActivationFunctionType.Silu)
                        nc.vector.tensor_mul(act[:, f, :], sil, gp_ps)
                    for po in range(ND):
                        wdt = wdpool.tile([128, NF2, 128], bf16, tag="wd")
                        nc.sync.dma_start(wdt, wd_d[s, po])
                        o_t = psopool.tile([128, TOK], f32, tag="o")
                        o_ps = o_t[:, :tn]
                        for f in range(NF2):
                            nc.tensor.matmul(o_ps, wdt[:, f, :], act[:, f, :],
                                             start=(f == 0), stop=(f == NF2 - 1))
                        ob_t = outpool.tile([128, TOK], f32, tag="ob")
                        ob = ob_t[:, :tn]
                        nc.vector.tensor_copy(ob, o_ps)
                        nc.sync.dma_start(out_d[:, po, ds(off, tn)], ob)
                    # Trickle in slot-B weights during phase A.
                    if s == 0 and t > 0 and bgroups:
                        n_emit = -(-len(GRP_SIZES) // max(1, len(tiles_a) - 1))
                        for _ in range(n_emit):
                            if bgroups:
                                load_wgrp(1, bgroups.pop(0))
                        if not bgroups and not htb0_loaded[0]:
                            nc.sync.dma_start(
                                htb0[:, :, :tiles_b[0]],
                                ht_d[:, :, ds(ca, tiles_b[0])])
                            htb0_loaded[0] = True
                    off += tn
                # Ensure all slot-B groups are loaded even for tiny phase A.
                if s == 0:
                    while bgroups:
                        load_wgrp(1, bgroups.pop(0))
    nc.compile()
    return nc


def _get_nc(caps):
    if caps not in _CACHE:
        _CACHE[caps] = _build(caps)
    return _CACHE[caps]


def _cap(n):
    return max(512, -(-n // 16) * 16)


def kernel(hidden_states, w_router, w_up, w_gate, w_down):
    h = np.asarray(hidden_states, np.float32).reshape(T, D)
    wr = np.asarray(w_router, np.float32)
    w_up = np.asarray(w_up, np.float32)
    w_gate = np.asarray(w_gate, np.float32)
    w_down = np.asarray(w_down, np.float32)

    # Router on host (fp32, same math as reference).
    logits = h @ wr                                            # [T, E]
    p = logits - logits.max(1, keepdims=True)
    np.exp(p, out=p)
    p /= p.sum(1, keepdims=True)
    sel = np.argsort(-p, axis=1, kind="stable")[:, :2]         # lax.top_k ties
    tix = np.arange(T)
    w1 = p[tix, sel[:, 0]]
    w2 = p[tix, sel[:, 1]]
    denom = w1 + w2
    rw1 = (w1 / denom).astype(np.float32)
    rw2 = (w2 / denom).astype(np.float32)

    # Token list + routing weight per expert.
    idx_e, wt_e = [], []
    for e in range(E):
        m = (sel[:, 0] == e) | (sel[:, 1] == e)
        idx = np.nonzero(m)[0]
        idx_e.append(idx)
        wt_e.append(np.where(sel[:, 0] == e, rw1, rw2)[idx].astype(np.float32))
    counts = np.array([len(i) for i in idx_e])

    # Pair heavy experts with light ones; each pair spans 2 cores (F halves).
    order = np.argsort(-counts, kind="stable")
    big, small = order[:4], order[4:][::-1]
    ca = _cap(int(counts[big].max()))
    cb = _cap(int(counts[small].max()))
    nc = _get_nc((ca, cb))

    # h^T in bf16, tiled [128, ND, T] (partition-inner on D).
    hT16 = np.ascontiguousarray(h.T).astype(BF16)              # [D, T]
    hT16 = hT16.reshape(ND, 128, T).transpose(1, 0, 2)         # [128, ND, T]

    def wslice(w, e, half):                                    # [D, F2] -> tiled
        ww = w[e][:, half * F2:(half + 1) * F2].astype(BF16)
        return ww.reshape(ND, 128, F2).transpose(1, 0, 2)      # [128, ND, F2]

    def wdslice(e, half):                                      # [F2, D] -> tiled
        ww = w_down[e][half * F2:(half + 1) * F2].astype(BF16)
        return ww.reshape(NF2, 128, ND, 128).transpose(2, 1, 0, 3)

    in_maps = []
    for pair in range(4):
        ea, eb = int(big[pair]), int(small[pair])
        ht_p = np.zeros((128, ND, ca + cb), BF16)
        ht_p[:, :, :counts[ea]] = hT16[:, :, idx_e[ea]]
        ht_p[:, :, ca:ca + counts[eb]] = hT16[:, :, idx_e[eb]]
        for half in range(2):
            in_maps.append({
                "ht": ht_p,
                "wu": np.ascontiguousarray(
                    np.stack([wslice(w_up, ea, half), wslice(w_up, eb, half)])),
                "wg": np.ascontiguousarray(
                    np.stack([wslice(w_gate, ea, half), wslice(w_gate, eb, half)])),
                "wd": np.ascontiguousarray(
                    np.stack([wdslice(ea, half), wdslice(eb, half)])),
            })

    res = run_bass_kernel_spmd(
        nc, in_maps, core_ids=list(range(E)),
        trace=TRACE, trace_cores=TRACE_CORES,
    )
    LAST["results"] = res

    out = np.zeros((T, D), np.float32)
    for pair in range(4):
        ea, eb = int(big[pair]), int(small[pair])
        oa = res.results[2 * pair]["out"]                      # [128, ND, ca+cb]
        obb = res.results[2 * pair + 1]["out"]
        o = (oa + obb).transpose(2, 1, 0).reshape(ca + cb, D)  # [tok, D]
        out[idx_e[ea]] += o[:counts[ea]] * wt_e[ea][:, None]
        out[idx_e[eb]] += o[ca:ca + counts[eb]] * wt_e[eb][:, None]

    return out.reshape(B, S, D), logits


# revision 12
# speedup vs baseline: 1.2885x; 1.0177x over previous
"""Mixtral sparse-MoE block on 8 TRN2 NeuronCores, expert-parallel.

Strategy (per sharding hint): the E=8 experts are sharded across the 8 cores
and the router is replicated on host (tiny: [8192,1024]x[1024,8]). Each
expert's FFN dim F is split in half across a core pair, and heavy-count
experts are paired with light ones so every core gets a near-equal token
load (top-2 routing, per-slot capacity). Each core runs two phases: slot-A
expert's half-FFN over its routed tokens, then slot-B's. The host applies
the renormalized routing weights and scatter-adds the partials (the "psum
of the combine").

Device math is bf16 matmul with fp32 PSUM accumulation; per-token expert
outputs are independent of batch composition, so gathered (sparse)
execution matches dense-then-mask execution numerically.
"""

import os
import sys

import numpy as np
import ml_dtypes

for _p in ("/opt/trn_rl_repo", "/root/.axon_site/_ro/trn_rl_repo"):
    if os.path.isdir(_p) and _p not in sys.path:
        sys.path.append(_p)

import concourse.mybir as mybir
import concourse.tile as tile
from concourse import bacc
from concourse.bass import ds, ts
from concourse.bass_utils import run_bass_kernel_spmd

BF16 = ml_dtypes.bfloat16

B, S, D, F, E = 2, 4096, 1024, 3584, 8
T = B * S
TOK = 512            # token tile (one PSUM bank of fp32)
ND = D // 128        # 8 contraction chunks for up/gate
F2 = F // 2          # FFN half per core
NF2 = F2 // 128      # 14 chunks for the down contraction

TRACE = False        # set by test.py to collect an NTFF profile
TRACE_CORES = None
LAST = {}            # test.py reads LAST["results"] (BassKernelResults)

_CACHE = {}

# Weight preload DMA granularity: fine groups up front so the first matmuls
# wait on ~1.5MB, coarser after.
GRP_SIZES = [2, 2, 2] + [4] * ((NF2 - 6) // 4)
assert sum(GRP_SIZES) == NF2


def _tiles_of(n):
    t = [TOK] * (n // TOK)
    if n % TOK:
        t.append(n % TOK)
    return t


def _build(caps):
    """One-core Bass program: two half-FFN SwiGLU phases (slot A, slot B).

    Inputs: ht [128,ND,CT] bf16      = h^T of slot-A tokens then slot-B tokens
            wu/wg [2,128,ND,F2] bf16 = slot's w_up/w_gate half (D on partitions)
            wd [2,ND,128,NF2,128] bf16 = slot's w_down half
    Output: out [128,ND,CT] f32 = unscaled half-FFN expert output^T
    """
    ca, cb = caps
    ct = ca + cb
    nc = bacc.Bacc("TRN2", target_bir_lowering=False, debug=False, num_devices=E)
    f32, bf16 = mybir.dt.float32, mybir.dt.bfloat16
    ht_d = nc.dram_tensor("ht", [128, ND, ct], bf16, kind="ExternalInput").ap()
    wu_d = nc.dram_tensor("wu", [2, 128, ND, F2], bf16, kind="ExternalInput").ap()
    wg_d = nc.dram_tensor("wg", [2, 128, ND, F2], bf16, kind="ExternalInput").ap()
    wd_d = nc.dram_tensor("wd", [2, ND, 128, NF2, 128], bf16,
                          kind="ExternalInput").ap()
    out_d = nc.dram_tensor("out", [128, ND, ct], f32, kind="ExternalOutput").ap()

    fbases = np.cumsum([0] + GRP_SIZES[:-1]).tolist()

    with tile.TileContext(nc) as tc:
        with (
            tc.tile_pool(name="wconst", bufs=1) as wpool,
            tc.tile_pool(name="htp", bufs=2) as htpool,
            tc.tile_pool(name="wdp", bufs=3) as wdpool,
            tc.tile_pool(name="actp", bufs=2) as actpool,
            tc.tile_pool(name="silp", bufs=2) as silpool,
            tc.tile_pool(name="outp", bufs=3) as outpool,
            tc.tile_pool(name="ps", bufs=3, space="PSUM") as pspool,
            tc.tile_pool(name="pso", bufs=2, space="PSUM") as psopool,
        ):
            wu_f = [[None] * NF2, [None] * NF2]
            wg_f = [[None] * NF2, [None] * NF2]

            def load_wgrp(s, g):
                gs, fbase = GRP_SIZES[g], fbases[g]
                wut = wpool.tile([128, ND, gs * 128], bf16, tag=f"wu{s}_{g}")
                nc.sync.dma_start(wut, wu_d[s][:, :, ds(fbase * 128, gs * 128)])
                wgt = wpool.tile([128, ND, gs * 128], bf16, tag=f"wg{s}_{g}")
                nc.sync.dma_start(wgt, wg_d[s][:, :, ds(fbase * 128, gs * 128)])
                for j in range(gs):
                    wu_f[s][fbase + j] = wut[:, :, ts(j, 128)]
                    wg_f[s][fbase + j] = wgt[:, :, ts(j, 128)]

            # Phase-A token tile 0 rhs first, then slot-A weights.
            tiles_a, tiles_b = _tiles_of(ca), _tiles_of(cb)
            ht0 = htpool.tile([128, ND, TOK], bf16, tag="ht")
            nc.sync.dma_start(ht0[:, :2, :tiles_a[0]], ht_d[:, :2, ds(0, tiles_a[0])])
            load_wgrp(0, 0)
            nc.sync.dma_start(ht0[:, 2:, :tiles_a[0]], ht_d[:, 2:, ds(0, tiles_a[0])])
            for g in range(1, len(GRP_SIZES)):
                load_wgrp(0, g)

            # Slot-B weight groups are emitted interleaved with phase A's tile
            # loop so they never queue ahead of phase-A token DMAs.
            bgroups = list(range(len(GRP_SIZES)))
            htb0 = htpool.tile([128, ND, TOK], bf16, tag="htb0")
            htb0_loaded = [False]

            def upgate(s, t, tn, off):
                """Up/gate phase of one token tile -> act tile (sliced)."""
                if s == 0 and t == 0:
                    ht = ht0[:, :, :tn]
                elif s == 1 and t == 0:
                    ht = htb0[:, :, :tn]
                    if not htb0_loaded[0]:
                        nc.sync.dma_start(ht, ht_d[:, :, ds(off, tn)])
                else:
                    ht_t = htpool.tile([128, ND, TOK], bf16, tag="ht")
                    ht = ht_t[:, :, :tn]
                    nc.sync.dma_start(ht, ht_d[:, :, ds(off, tn)])
                act_t = actpool.tile([128, NF2, TOK], bf16, tag="act")
                act = act_t[:, :, :tn]
                for f in range(NF2):
                    up_t = pspool.tile([128, TOK], f32, tag="up")
                    up_ps = up_t[:, :tn]
                    gp_t = pspool.tile([128, TOK], f32, tag="gp")
                    gp_ps = gp_t[:, :tn]
                    for d in range(ND):
                        nc.tensor.matmul(up_ps, wu_f[s][f][:, d, :], ht[:, d, :],
                                         start=(d == 0), stop=(d == ND - 1))
                    for d in range(ND):
                        nc.tensor.matmul(gp_ps, wg_f[s][f][:, d, :], ht[:, d, :],
                                         start=(d == 0), stop=(d == ND - 1))
                    sil_t = silpool.tile([128, TOK], f32, tag="sil")
                    sil = sil_t[:, :tn]
                    nc.scalar.activation(sil, up_ps,
                                         mybir.ActivationFunctionType.Silu)
                    nc.vector.tensor_mul(act[:, f, :], sil, gp_ps)
                return act

            def down(s, acts):
                """Down phase for 1-2 token tiles sharing each wd load."""
                for po in range(ND):
                    wdt = wdpool.tile([128, NF2, 128], bf16, tag="wd")
                    nc.sync.dma_start(wdt, wd_d[s, po])
                    for act, tn, off in acts:
                        o_t = psopool.tile([128, TOK], f32, tag="o")
                        o_ps = o_t[:, :tn]
                        for f in range(NF2):
                            nc.tensor.matmul(o_ps, wdt[:, f, :], act[:, f, :],
                                             start=(f == 0), stop=(f == NF2 - 1))
                        ob_t = outpool.tile([128, TOK], f32, tag="ob")
                        ob = ob_t[:, :tn]
                        nc.vector.tensor_copy(ob, o_ps)
                        nc.sync.dma_start(out_d[:, po, ds(off, tn)], ob)

            def trickle_b(t):
                if bgroups:
                    n_emit = -(-len(GRP_SIZES) // max(1, len(tiles_a) - 1))
                    for _ in range(n_emit):
                        if bgroups:
                            load_wgrp(1, bgroups.pop(0))
                    if not bgroups and not htb0_loaded[0]:
                        nc.sync.dma_start(htb0[:, :, :tiles_b[0]],
                                          ht_d[:, :, ds(ca, tiles_b[0])])
                        htb0_loaded[0] = True

            off = 0
            for s, tiles in ((0, tiles_a), (1, tiles_b)):
                t = 0
                while t < len(tiles):
                    pair = tiles[t:t + 2]
                    acts = []
                    for j, tn in enumerate(pair):
                        acts.append((upgate(s, t + j, tn, off), tn, off))
                        off += tn
                        if s == 0 and t + j > 0:
                            trickle_b(t + j)
                    down(s, acts)
                    t += len(pair)
                # Ensure all slot-B groups are loaded even for tiny phase A.
                if s == 0:
                    while bgroups:
                        load_wgrp(1, bgroups.pop(0))
                    if not htb0_loaded[0]:
                        nc.sync.dma_start(htb0[:, :, :tiles_b[0]],
                                          ht_d[:, :, ds(ca, tiles_b[0])])
                        htb0_loaded[0] = True
    nc.compile()
    return nc


def _get_nc(caps):
    if caps not in _CACHE:
        _CACHE[caps] = _build(caps)
    return _CACHE[caps]


def _cap(n):
    return max(512, -(-n // 16) * 16)


def kernel(hidden_states, w_router, w_up, w_gate, w_down):
    h = np.asarray(hidden_states, np.float32).reshape(T, D)
    wr = np.asarray(w_router, np.float32)
    w_up = np.asarray(w_up, np.float32)
    w_gate = np.asarray(w_gate, np.float32)
    w_down = np.asarray(w_down, np.float32)

    # Router on host (fp32, same math as reference).
    logits = h @ wr                                            # [T, E]
    p = logits - logits.max(1, keepdims=True)
    np.exp(p, out=p)
    p /= p.sum(1, keepdims=True)
    sel = np.argsort(-p, axis=1, kind="stable")[:, :2]         # lax.top_k ties
    tix = np.arange(T)
    w1 = p[tix, sel[:, 0]]
    w2 = p[tix, sel[:, 1]]
    denom = w1 + w2
    rw1 = (w1 / denom).astype(np.float32)
    rw2 = (w2 / denom).astype(np.float32)

    # Token list + routing weight per expert.
    idx_e, wt_e = [], []
    for e in range(E):
        m = (sel[:, 0] == e) | (sel[:, 1] == e)
        idx = np.nonzero(m)[0]
        idx_e.append(idx)
        wt_e.append(np.where(sel[:, 0] == e, rw1, rw2)[idx].astype(np.float32))
    counts = np.array([len(i) for i in idx_e])

    # Pair heavy experts with light ones; each pair spans 2 cores (F halves).
    order = np.argsort(-counts, kind="stable")
    big, small = order[:4], order[4:][::-1]
    ca = _cap(int(counts[big].max()))
    cb = _cap(int(counts[small].max()))
    nc = _get_nc((ca, cb))

    # h^T in bf16, tiled [128, ND, T] (partition-inner on D).
    hT16 = np.ascontiguousarray(h.T).astype(BF16)              # [D, T]
    hT16 = hT16.reshape(ND, 128, T).transpose(1, 0, 2)         # [128, ND, T]

    def wslice(w, e, half):                                    # [D, F2] -> tiled
        ww = w[e][:, half * F2:(half + 1) * F2].astype(BF16)
        return ww.reshape(ND, 128, F2).transpose(1, 0, 2)      # [128, ND, F2]

    def wdslice(e, half):                                      # [F2, D] -> tiled
        ww = w_down[e][half * F2:(half + 1) * F2].astype(BF16)
        return ww.reshape(NF2, 128, ND, 128).transpose(2, 1, 0, 3)

    in_maps = []
    for pair in range(4):
        ea, eb = int(big[pair]), int(small[pair])
        ht_p = np.zeros((128, ND, ca + cb), BF16)
        ht_p[:, :, :counts[ea]] = hT16[:, :, idx_e[ea]]
        ht_p[:, :, ca:ca + counts[eb]] = hT16[:, :, idx_e[eb]]
        for half in range(2):
            in_maps.append({
                "ht": ht_p,
                "wu": np.ascontiguousarray(
                    np.stack([wslice(w_up, ea, half), wslice(w_up, eb, half)])),
                "wg": np.ascontiguousarray(
                    np.stack([wslice(w_gate, ea, half), wslice(w_gate, eb, half)])),
                "wd": np.ascontiguousarray(
                    np.stack([wdslice(ea, half), wdslice(eb, half)])),
            })

    res = run_bass_kernel_spmd(
        nc, in_maps, core_ids=list(range(E)),
        trace=TRACE, trace_cores=TRACE_CORES,
    )
    LAST["results"] = res

    out = np.zeros((T, D), np.float32)
    for pair in range(4):
        ea, eb = int(big[pair]), int(small[pair])
        oa = res.results[2 * pair]["out"]                      # [128, ND, ca+cb]
        obb = res.results[2 * pair + 1]["out"]
        o = (oa + obb).transpose(2, 1, 0).reshape(ca + cb, D)  # [tok, D]
        out[idx_e[ea]] += o[:counts[ea]] * wt_e[ea][:, None]
        out[idx_e[eb]] += o[ca:ca + counts[eb]] * wt_e[eb][:, None]

    return out.reshape(B, S, D), logits
